# revision 47
# baseline (speedup 1.0000x reference)
"""Trainium2 Bass kernel for nn_Cache_65627100283720 (retrieval_knn).

Reference computation (jax):
    q = query.transpose(0,2,1,3).reshape(QL, B, L*H) @ W_summary.T + b_summary
    scores  = einsum('ibd,bnd->ibn', q, keys.transpose(1,0,2)) / sqrt(DK)
    weights = softmax(scores, -1)                      # -> attention [QL*B,1,N]
    topk_indices = top_k(weights, 16).T                # -> [16, QL*B]
(The big `values` einsum in the reference is dead code - its result is
discarded - so `values` never touches the device.)

Sharding (8 cores): the dominant cost is reading W_summary (32 MiB) and query
(16 MiB). We shard the L*H=16384 contraction dim: core c owns L-steps
[4c,4c+4) i.e. a 2048-slice, reads only W^T[2048c:2048c+2048] (4 MiB) and its
query slice (2 MiB), and computes a partial q-projection [64,512] (launch 1).
The host then re-shards those partials by batch (row order is b-major, so a
row-chunk == a batch shard) - pure gather/layout glue, no arithmetic - and
launch 2 sums the 8 partials on-device (3 tree adds), adds the bias, and runs
scores, softmax and top-16 (Max8/MatchReplace/FindIndex8) per core for its 2
batches. (A single-launch ReduceScatter design is blocked: intra-chip
collectives crash NRT under the axon/PJRT execution path.)

Perf notes (vs the instruction cost model):
- all host-side layouts are packed partition-outer so every DMA line is a
  >=1 KiB contiguous run (full HBM bandwidth);
- a few throwaway matmuls warm the PE clock (HAM ramp) during the initial
  DMA window so the real fp32 matmuls run at full rate;
- top-16 runs on the raw scores (same order as softmax weights - softmax is
  monotone) so the DVE top-k chain overlaps the ACT softmax chain.
"""

import sys
import time

if "/opt/trn_rl_repo" not in sys.path:
    sys.path.insert(0, "/opt/trn_rl_repo")

import ml_dtypes
import numpy as np

import concourse.bacc as bacc
import concourse.mybir as mybir
import concourse.tile as tile
from concourse.bass_utils import run_bass_kernel_spmd

NCORES = 8
QL, L, B, H = 4, 32, 16, 512
N, DK = 128, 512
LH = L * H                 # 16384
KC = LH // NCORES          # 2048 contraction elems per core
KT = KC // 128             # 16 k-tiles per core
ROWS = QL * B              # 64 (row order: r = b*QL + i, b-major)
BPC = B // NCORES          # 2 batches per core
RPC = QL * BPC             # 8 rows per core
DKT = DK // 128            # 4 dk-tiles
TOPK = 16
SCALE = 1.0 / np.sqrt(np.float32(DK))
WPAIR = 2                  # k-tiles per W DMA transfer
NEG = -1.0e30              # below any score

F32 = mybir.dt.float32
BF16 = mybir.dt.bfloat16
U32 = mybir.dt.uint32

_CACHE = {}  # (which, niter) -> compiled Bacc program


def _body_iter(tc, niter):
    """niter >= 0: python-unrolled; niter < 0: tc.For_i hardware loop of
    -niter iterations (for loop-delta HW timing)."""
    if niter >= 0:
        yield from range(niter)
    else:
        with tc.For_i(0, -niter, 1):
            yield 0


def _warmup_pe(nc, pool, psum, n_mm=5, n_small=0):
    """Throwaway matmuls to ramp the PE clock while input DMAs run."""
    junk = pool.tile([128, N], F32, tag="warm_junk")
    nc.vector.memset(junk, 0.0)
    wps = psum.tile([128, N], F32, tag="warm_ps")
    for _ in range(n_mm):
        nc.tensor.matmul(wps, lhsT=junk[:], rhs=junk[:], start=True, stop=True)
    for _ in range(n_small):
        nc.tensor.matmul(
            wps[:, 0:64], lhsT=junk[:], rhs=junk[:, 0:64], start=True, stop=True
        )


def _emit_k1(nc, niter=1):
    """Partial projection: qp[64,512] = X_c^T-tiles^T @ W_c^T-tiles.

    fp32 operands are split hi+lo into bf16 on the host; each k-tile does
    3 full-rate bf16 passes (hi*hi + hi*lo + lo*hi, fp32 PSUM accumulate)
    instead of one quarter-rate fp32 matmul. Dropped lo*lo term is
    ~2^-16 relative - far below the fp32 path's own rounding noise."""
    # xt[h][g, p, k, r] (k-half groups g), wt[h][j, p, i, d]; h = hi/lo
    XG = 2  # xt k-groups
    KG = KT // XG
    xt_d = [
        nc.dram_tensor(f"xt{h}", [XG, 128, KG, ROWS], BF16, kind="ExternalInput")
        for h in range(2)
    ]
    wt_d = [
        nc.dram_tensor(
            f"wt{h}", [KT // WPAIR, 128, WPAIR, DK], BF16, kind="ExternalInput"
        )
        for h in range(2)
    ]
    qp_d = nc.dram_tensor("qp", [ROWS, DK], F32, kind="ExternalOutput")

    with tile.TileContext(nc) as tc:
        with (
            tc.tile_pool(name="xpool", bufs=4) as xpool,
            tc.tile_pool(name="wpool", bufs=16) as wpool,
            tc.tile_pool(name="opool", bufs=2) as opool,
            tc.tile_pool(name="psum", bufs=2, space="PSUM") as psum,
        ):
            _warmup_pe(nc, opool, psum, n_mm=5, n_small=6)
            for _ in _body_iter(tc, niter):
                xt_sb = {}

                def load_xt(g, h):
                    x_h = xpool.tile([128, KG, ROWS], BF16, tag=f"xt{h}")
                    nc.sync.dma_start(out=x_h, in_=xt_d[h][g])
                    xt_sb[h, g] = x_h

                load_xt(0, 0)
                qp_ps = psum.tile([ROWS, DK], F32, tag="qp")
                nmm = 0
                for j in range(KT // WPAIR):
                    wt_j = []
                    for h in range(2):
                        w_h = wpool.tile([128, WPAIR, DK], BF16, tag=f"wt{h}")
                        nc.sync.dma_start(out=w_h, in_=wt_d[h][j])
                        wt_j.append(w_h)
                        if j == 0 and h == 0:
                            # xt-lo lands after wt0-hi: the hi*hi pass can
                            # start as soon as xt-hi + wt0-hi are in
                            load_xt(0, 1)
                    if j == 2:
                        # group-b xt arrives behind W pairs 0-2, ahead of its
                        # first use at k = KG (pair KG/WPAIR)
                        load_xt(1, 0)
                        load_xt(1, 1)
                    for i in range(WPAIR):
                        k = j * WPAIR + i
                        g, kg = divmod(k, KG)
                        # hi*hi first: it only needs the hi transfers
                        for xh, wh in ((0, 0), (0, 1), (1, 0)):
                            nc.tensor.matmul(
                                qp_ps,
                                lhsT=xt_sb[xh, g][:, kg, :],
                                rhs=wt_j[wh][:, i, :],
                                start=(nmm == 0),
                                stop=(nmm == 3 * KT - 1),
                            )
                            nmm += 1
                qp_sb = opool.tile([ROWS, DK], F32, tag="qp_sb")
                nc.vector.tensor_copy(qp_sb, qp_ps)
                nc.sync.dma_start(out=qp_d.ap(), in_=qp_sb)
    nc.compile()
    return nc


def _emit_k2(nc, niter=1):
    """Sum 8 partials + bias (already transposed by host glue), scores,
    softmax, top-16 for this core's 2 batches."""
    NSRC = NCORES + 1  # block 0 = bias, blocks 1..8 = per-core partials
    TRW = DKT * RPC    # one source block: (t, r) columns
    # parts[p, s, t, r]: dk%128 p, source block s, dk-tile t, row r
    parts_d = nc.dram_tensor(
        "parts", [128, NSRC, DKT, RPC], F32, kind="ExternalInput"
    )
    kt_d = nc.dram_tensor("kt", [128, BPC, DKT, N], F32, kind="ExternalInput")
    attn_d = nc.dram_tensor("attn", [RPC, N], F32, kind="ExternalOutput")
    idx_d = nc.dram_tensor("idx", [RPC, TOPK], U32, kind="ExternalOutput")

    with tile.TileContext(nc) as tc:
        with (
            tc.tile_pool(name="kpool", bufs=1) as kpool,
            tc.tile_pool(name="small", bufs=2) as small,
            tc.tile_pool(name="psum", bufs=2, space="PSUM") as psum,
        ):
            _warmup_pe(nc, small, psum, n_mm=5, n_small=8)
            kt_sb = kpool.tile([128, BPC, DKT, N], F32, tag="kt")

            for it in _body_iter(tc, niter):
                # qt layout: [128 (dk%128), (t, r)] with col = t*RPC + r
                parts_sb = small.tile([128, NSRC * TRW], F32, tag="parts")
                nc.sync.dma_start(
                    out=parts_sb[:].rearrange(
                        "p (s t r) -> p s t r", s=NSRC, t=DKT
                    ),
                    in_=parts_d.ap(),
                )
                if it == 0:
                    for b in range(BPC):
                        nc.sync.dma_start(
                            out=kt_sb[:, b], in_=kt_d[:, b]
                        )

                # tree-sum source blocks 1..8, then fold in bias block 0
                def blk(i, j):
                    return parts_sb[:, i * TRW : j * TRW]

                nc.vector.tensor_add(blk(1, 5), blk(1, 5), blk(5, 9))
                nc.vector.tensor_add(blk(1, 3), blk(1, 3), blk(3, 5))
                nc.vector.tensor_add(blk(1, 2), blk(1, 2), blk(2, 3))
                qt_sb = small.tile([128, TRW], F32, tag="qt")
                nc.vector.tensor_add(qt_sb, blk(0, 1), blk(1, 2))

                # scores for both batches into one [4, 2*128] PSUM bank
                i16_all = small.tile([QL, BPC, TOPK], U32, tag="i16")
                s_ps = psum.tile([QL, BPC, N], F32, tag="s_ps")
                for b in range(BPC):
                    for t in range(DKT):
                        nc.tensor.matmul(
                            s_ps[:, b, :],
                            lhsT=qt_sb[
                                :, t * RPC + b * QL : t * RPC + (b + 1) * QL
                            ],
                            rhs=kt_sb[:, b, t, :],
                            start=(t == 0),
                            stop=(t == DKT - 1),
                        )

                # s2 = s - rowmax (per batch segment), staged into SBUF; this
                # both matches the reference's stabilized softmax and makes
                # the DVE top-k ops read SBUF (cheaper than PSUM reads).
                m_sb = small.tile([QL, BPC], F32, tag="m")
                nc.vector.reduce_max(m_sb, s_ps, axis=mybir.AxisListType.X)
                s2 = small.tile([QL, BPC, N], F32, tag="s2")
                nc.vector.tensor_sub(
                    s2,
                    s_ps,
                    m_sb[:]
                    .rearrange("p (b one) -> p b one", one=1)
                    .to_broadcast([QL, BPC, N]),
                )

                # top-16 per batch on s2 (same order as softmax weights)
                for b in range(BPC):
                    v16 = small.tile([QL, TOPK], F32, tag=f"v16_{b}")
                    w2 = small.tile([QL, N], F32, tag=f"w2_{b}")
                    nc.vector.max(out=v16[:, 0:8], in_=s2[:, b, :])
                    nc.vector.max_index(
                        i16_all[:, b, 0:8], v16[:, 0:8], s2[:, b, :]
                    )
                    nc.vector.match_replace(
                        out=w2,
                        in_to_replace=v16[:, 0:8],
                        in_values=s2[:, b, :],
                        imm_value=NEG,
                    )
                    nc.vector.max(out=v16[:, 8:16], in_=w2)
                    nc.vector.max_index(i16_all[:, b, 8:16], v16[:, 8:16], w2)
                nc.sync.dma_start(
                    out=idx_d.ap().rearrange("(b i) k -> i b k", b=BPC),
                    in_=i16_all,
                )

                # softmax tail: exp, per-batch sum, normalize
                w_sb = small.tile([QL, BPC, N], F32, tag="w")
                nc.scalar.activation(
                    w_sb,
                    s2,
                    mybir.ActivationFunctionType.Exp,
                    scale=float(SCALE),
                )
                sum_sb = small.tile([QL, BPC], F32, tag="sum")
                nc.vector.reduce_sum(sum_sb, w_sb, axis=mybir.AxisListType.X)
                rec = small.tile([QL, BPC], F32, tag="rec")
                nc.vector.reciprocal(rec, sum_sb)
                wn_all = small.tile([QL, BPC, N], F32, tag="wn")
                nc.vector.tensor_mul(
                    wn_all,
                    w_sb,
                    rec[:]
                    .rearrange("p (b one) -> p b one", one=1)
                    .to_broadcast([QL, BPC, N]),
                )
                nc.sync.dma_start(
                    out=attn_d.ap().rearrange("(b i) n -> i b n", b=BPC),
                    in_=wn_all,
                )
    nc.compile()
    return nc


def _get_nc(which, niter=1):
    key = (which, niter)
    if key not in _CACHE:
        nc = bacc.Bacc(
            trn_type="TRN2",
            target_bir_lowering=False,
            debug=False,
            num_devices=NCORES,
        )
        _CACHE[key] = (_emit_k1 if which == "k1" else _emit_k2)(nc, niter)
    return _CACHE[key]


def _split_bf16(a):
    """fp32 -> (hi, lo) bf16 pair with hi + lo ~= a (|err| <~ 2^-16 |a|)."""
    hi = a.astype(ml_dtypes.bfloat16)
    lo = (a - hi.astype(np.float32)).astype(ml_dtypes.bfloat16)
    return hi, lo


def _k1_in_maps(query, W_summary):
    # X[r, lh]: r = b*QL + i (b-major), lh = l*H + h
    x = np.ascontiguousarray(
        np.asarray(query, dtype=np.float32).transpose(2, 0, 1, 3)
    ).reshape(ROWS, LH)
    xt = np.ascontiguousarray(x.T)                     # [LH, 64]
    wt = np.ascontiguousarray(np.asarray(W_summary, dtype=np.float32).T)  # [LH, DK]
    xt_hl = _split_bf16(xt)
    wt_hl = _split_bf16(wt)
    in_maps = []
    for c in range(NCORES):
        m = {}
        for h in range(2):
            # xt packed partition-outer: [128, KT, ROWS]
            m[f"xt{h}"] = np.ascontiguousarray(
                xt_hl[h][c * KC : (c + 1) * KC]
                .reshape(2, KT // 2, 128, ROWS)
                .transpose(0, 2, 1, 3)
            )
            # wt packed partition-outer per WPAIR group
            m[f"wt{h}"] = np.ascontiguousarray(
                wt_hl[h][c * KC : (c + 1) * KC]
                .reshape(KT // WPAIR, WPAIR, 128, DK)
                .transpose(0, 2, 1, 3)
            )
        in_maps.append(m)
    return in_maps


def _k2_in_maps(qps, keys, b_summary):
    """qps: list of 8 per-core partial projections [64, 512] from launch 1.
    Pure gather/transpose glue - no arithmetic on values."""
    kt = np.ascontiguousarray(
        np.asarray(keys, dtype=np.float32).transpose(1, 2, 0)
    )  # [B, DK, N]
    bias = np.asarray(b_summary, dtype=np.float32)
    # bias block [p, t, r] = bias[t*128 + p], replicated over r
    biast = np.broadcast_to(
        bias.reshape(DKT, 128).T.reshape(128, 1, DKT, 1), (128, 1, DKT, RPC)
    )
    in_maps = []
    for c in range(NCORES):
        # parts[p, s, t, r]; s=0 is the bias block, s=1.. the partials
        srcs = np.stack(
            [
                np.asarray(qps[s])[c * RPC : (c + 1) * RPC].T.reshape(
                    DKT, 128, RPC
                )
                for s in range(NCORES)
            ]
        ).transpose(2, 0, 1, 3)
        parts = np.ascontiguousarray(
            np.concatenate([biast, srcs], axis=1)
        )
        # kt packed partition-outer: [128, BPC, DKT, N]
        ktc = np.ascontiguousarray(
            kt[c * BPC : (c + 1) * BPC]
            .reshape(BPC, DKT, 128, N)
            .transpose(2, 0, 1, 3)
        )
        in_maps.append({"parts": parts, "kt": ktc})
    return in_maps


def _unshard(results):
    """Per-core [RPC,N]/[RPC,TOPK] (row r'=b_local*QL+i) -> full outputs."""
    attn = np.empty((QL, B, N), dtype=np.float32)
    idx = np.empty((QL, B, TOPK), dtype=np.int32)
    for c in range(NCORES):
        a = np.asarray(results[c]["attn"]).reshape(BPC, QL, N)
        t = np.asarray(results[c]["idx"]).view(np.int32).reshape(BPC, QL, TOPK)
        attn[:, c * BPC : (c + 1) * BPC, :] = a.transpose(1, 0, 2)
        idx[:, c * BPC : (c + 1) * BPC, :] = t.transpose(1, 0, 2)
    attention = attn.reshape(ROWS, 1, N)
    topk_indices = np.ascontiguousarray(idx.reshape(ROWS, TOPK).T)
    return attention, topk_indices


def _run_spmd(nc, in_maps, tries=3):
    """run_bass_kernel_spmd with retry: the axon/PJRT execution path
    occasionally throws a transient NRT_EXEC_UNIT_UNRECOVERABLE on the
    first NEFF dispatch of a session; a backend reset + retry heals it."""
    for attempt in range(tries):
        try:
            return run_bass_kernel_spmd(
                nc, in_maps, core_ids=list(range(NCORES))
            )
        except Exception:
            if attempt == tries - 1:
                raise
            try:
                import jax

                jax.clear_caches()
                jax.clear_backends()
            except Exception:
                pass
            time.sleep(2.0)


def kernel(query, keys, values, W_summary, b_summary):
    del values  # dead in the reference: its einsum result is discarded
    res1 = _run_spmd(_get_nc("k1"), _k1_in_maps(query, W_summary))
    qps = [res1.results[c]["qp"] for c in range(NCORES)]
    res2 = _run_spmd(_get_nc("k2"), _k2_in_maps(qps, keys, b_summary))
    return _unshard(res2.results)


# revision 51
# speedup vs baseline: 1.0183x; 1.0183x over previous
"""Trainium2 Bass kernel for nn_Cache_65627100283720 (retrieval_knn).

Reference computation (jax):
    q = query.transpose(0,2,1,3).reshape(QL, B, L*H) @ W_summary.T + b_summary
    scores  = einsum('ibd,bnd->ibn', q, keys.transpose(1,0,2)) / sqrt(DK)
    weights = softmax(scores, -1)                      # -> attention [QL*B,1,N]
    topk_indices = top_k(weights, 16).T                # -> [16, QL*B]
(The big `values` einsum in the reference is dead code - its result is
discarded - so `values` never touches the device.)

Sharding (8 cores): the dominant cost is reading W_summary (32 MiB) and query
(16 MiB). We shard the L*H=16384 contraction dim: core c owns L-steps
[4c,4c+4) i.e. a 2048-slice, reads only W^T[2048c:2048c+2048] (4 MiB) and its
query slice (2 MiB), and computes a partial q-projection [64,512] (launch 1).
The host then re-shards those partials by batch (row order is b-major, so a
row-chunk == a batch shard) - pure gather/layout glue, no arithmetic - and
launch 2 sums the 8 partials on-device (3 tree adds), adds the bias, and runs
scores, softmax and top-16 (Max8/MatchReplace/FindIndex8) per core for its 2
batches. (A single-launch ReduceScatter design is blocked: intra-chip
collectives crash NRT under the axon/PJRT execution path.)

Perf notes (vs the instruction cost model; modeled ~22us + ~13us):
- the projection runs as bf16 hi+lo 3-pass matmuls (full PE rate, ~2^-16
  error) instead of quarter-rate fp32; precision suffices to reproduce the
  reference's top-k ordering exactly;
- all host-side layouts are packed partition-outer so every DMA line is a
  >=1 KiB contiguous run (full HBM bandwidth), and launch 1 is DMA-bound at
  ~13us (4.5 MiB/core at ~352 GB/s);
- a few throwaway matmuls warm the PE clock (HAM ramp) during the initial
  DMA window so the real matmuls run at 2.4 GHz;
- top-16 runs on the max-shifted scores (same order as softmax weights -
  softmax is monotone) so the DVE top-k chain overlaps the ACT exp chain.
"""

import os
import sys
import time

if "/opt/trn_rl_repo" not in sys.path:
    sys.path.insert(0, "/opt/trn_rl_repo")

import ml_dtypes
import numpy as np

import concourse.bacc as bacc
import concourse.mybir as mybir
import concourse.tile as tile
from concourse.bass_utils import run_bass_kernel_spmd

NCORES = 8
QL, L, B, H = 4, 32, 16, 512
N, DK = 128, 512
LH = L * H                 # 16384
KC = LH // NCORES          # 2048 contraction elems per core
KT = KC // 128             # 16 k-tiles per core
ROWS = QL * B              # 64 (row order: r = b*QL + i, b-major)
BPC = B // NCORES          # 2 batches per core
RPC = QL * BPC             # 8 rows per core
DKT = DK // 128            # 4 dk-tiles
TOPK = 16
SCALE = 1.0 / np.sqrt(np.float32(DK))
WPAIR = 2                  # k-tiles per W DMA transfer
NEG = -1.0e30              # below any score

F32 = mybir.dt.float32
BF16 = mybir.dt.bfloat16
U32 = mybir.dt.uint32

_CACHE = {}  # (which, niter) -> compiled Bacc program


def _body_iter(tc, niter):
    """niter >= 0: python-unrolled; niter < 0: tc.For_i hardware loop of
    -niter iterations (for loop-delta HW timing)."""
    if niter >= 0:
        yield from range(niter)
    else:
        with tc.For_i(0, -niter, 1):
            yield 0


def _warmup_pe(nc, pool, psum, n_mm=5, n_small=0):
    """Throwaway matmuls to ramp the PE clock while input DMAs run."""
    junk = pool.tile([128, N], F32, tag="warm_junk")
    nc.vector.memset(junk, 0.0)
    wps = psum.tile([128, N], F32, tag="warm_ps")
    for _ in range(n_mm):
        nc.tensor.matmul(wps, lhsT=junk[:], rhs=junk[:], start=True, stop=True)
    for _ in range(n_small):
        nc.tensor.matmul(
            wps[:, 0:64], lhsT=junk[:], rhs=junk[:, 0:64], start=True, stop=True
        )


def _emit_k1(nc, niter=1):
    """Partial projection: qp[64,512] = X_c^T-tiles^T @ W_c^T-tiles.

    fp32 operands are split hi+lo into bf16 on the host; each k-tile does
    3 full-rate bf16 passes (hi*hi + hi*lo + lo*hi, fp32 PSUM accumulate)
    instead of one quarter-rate fp32 matmul. Dropped lo*lo term is
    ~2^-16 relative - far below the fp32 path's own rounding noise."""
    # xt[h][g, p, k, r] (k-half groups g), wt[h][j, p, i, d]; h = hi/lo
    XG = 2  # xt k-groups
    KG = KT // XG
    xt_d = [
        nc.dram_tensor(f"xt{h}", [XG, 128, KG, ROWS], BF16, kind="ExternalInput")
        for h in range(2)
    ]
    wt_d = [
        nc.dram_tensor(
            f"wt{h}", [KT // WPAIR, 128, WPAIR, DK], BF16, kind="ExternalInput"
        )
        for h in range(2)
    ]
    qp_d = nc.dram_tensor("qp", [ROWS, DK], F32, kind="ExternalOutput")

    with tile.TileContext(nc) as tc:
        with (
            tc.tile_pool(name="xpool", bufs=4) as xpool,
            tc.tile_pool(name="wpool", bufs=16) as wpool,
            tc.tile_pool(name="opool", bufs=2) as opool,
            tc.tile_pool(name="psum", bufs=2, space="PSUM") as psum,
        ):
            _warmup_pe(nc, opool, psum, n_mm=5)
            for _ in _body_iter(tc, niter):
                xt_sb = {}

                def load_xt(g, h):
                    x_h = xpool.tile([128, KG, ROWS], BF16, tag=f"xt{h}")
                    nc.sync.dma_start(out=x_h, in_=xt_d[h][g])
                    xt_sb[h, g] = x_h

                load_xt(0, 0)
                qp_ps = psum.tile([ROWS, DK], F32, tag="qp")
                nmm = 0
                for j in range(KT // WPAIR):
                    wt_j = []
                    for h in range(2):
                        w_h = wpool.tile([128, WPAIR, DK], BF16, tag=f"wt{h}")
                        nc.sync.dma_start(out=w_h, in_=wt_d[h][j])
                        wt_j.append(w_h)
                        if j == 0 and h == 0:
                            # xt-lo lands after wt0-hi: the hi*hi pass can
                            # start as soon as xt-hi + wt0-hi are in
                            load_xt(0, 1)
                    if j == 2:
                        # group-b xt arrives behind W pairs 0-2, ahead of its
                        # first use at k = KG (pair KG/WPAIR)
                        load_xt(1, 0)
                        load_xt(1, 1)
                    for i in range(WPAIR):
                        k = j * WPAIR + i
                        g, kg = divmod(k, KG)
                        # hi*hi first: it only needs the hi transfers
                        for xh, wh in ((0, 0), (0, 1), (1, 0)):
                            nc.tensor.matmul(
                                qp_ps,
                                lhsT=xt_sb[xh, g][:, kg, :],
                                rhs=wt_j[wh][:, i, :],
                                start=(nmm == 0),
                                stop=(nmm == 3 * KT - 1),
                            )
                            nmm += 1
                qp_sb = opool.tile([ROWS, DK], F32, tag="qp_sb")
                nc.vector.tensor_copy(qp_sb, qp_ps)
                nc.sync.dma_start(out=qp_d.ap(), in_=qp_sb)
    nc.compile()
    return nc


def _emit_k2(nc, niter=1):
    """Sum 8 partials + bias (already transposed by host glue), scores,
    softmax, top-16 for this core's 2 batches."""
    NSRC = NCORES + 1  # block 0 = bias, blocks 1..8 = per-core partials
    TRW = DKT * RPC    # one source block: (t, r) columns
    # parts[p, s, t, r]: dk%128 p, source block s, dk-tile t, row r
    parts_d = nc.dram_tensor(
        "parts", [128, NSRC, DKT, RPC], F32, kind="ExternalInput"
    )
    kt_d = nc.dram_tensor("kt", [128, BPC, DKT, N], F32, kind="ExternalInput")
    attn_d = nc.dram_tensor("attn", [RPC, N], F32, kind="ExternalOutput")
    idx_d = nc.dram_tensor("idx", [RPC, TOPK], U32, kind="ExternalOutput")

    with tile.TileContext(nc) as tc:
        with (
            tc.tile_pool(name="kpool", bufs=1) as kpool,
            tc.tile_pool(name="small", bufs=2) as small,
            tc.tile_pool(name="psum", bufs=2, space="PSUM") as psum,
        ):
            _warmup_pe(nc, small, psum, n_mm=5)
            kt_sb = kpool.tile([128, BPC, DKT, N], F32, tag="kt")

            for it in _body_iter(tc, niter):
                # qt layout: [128 (dk%128), (t, r)] with col = t*RPC + r
                parts_sb = small.tile([128, NSRC * TRW], F32, tag="parts")
                nc.sync.dma_start(
                    out=parts_sb[:].rearrange(
                        "p (s t r) -> p s t r", s=NSRC, t=DKT
                    ),
                    in_=parts_d.ap(),
                )
                if it == 0:
                    for b in range(BPC):
                        nc.sync.dma_start(
                            out=kt_sb[:, b], in_=kt_d[:, b]
                        )

                # tree-sum source blocks 1..8, then fold in bias block 0
                def blk(i, j):
                    return parts_sb[:, i * TRW : j * TRW]

                nc.vector.tensor_add(blk(1, 5), blk(1, 5), blk(5, 9))
                nc.vector.tensor_add(blk(1, 3), blk(1, 3), blk(3, 5))
                nc.vector.tensor_add(blk(1, 2), blk(1, 2), blk(2, 3))
                qt_sb = small.tile([128, TRW], F32, tag="qt")
                nc.vector.tensor_add(qt_sb, blk(0, 1), blk(1, 2))

                # scores for both batches into one [4, 2*128] PSUM bank
                i16_all = small.tile([QL, BPC, TOPK], U32, tag="i16")
                s_ps = psum.tile([QL, BPC, N], F32, tag="s_ps")
                for b in range(BPC):
                    for t in range(DKT):
                        nc.tensor.matmul(
                            s_ps[:, b, :],
                            lhsT=qt_sb[
                                :, t * RPC + b * QL : t * RPC + (b + 1) * QL
                            ],
                            rhs=kt_sb[:, b, t, :],
                            start=(t == 0),
                            stop=(t == DKT - 1),
                        )

                # s2 = s - rowmax (per batch segment), staged into SBUF; this
                # both matches the reference's stabilized softmax and makes
                # the DVE top-k ops read SBUF (cheaper than PSUM reads).
                m_sb = small.tile([QL, BPC], F32, tag="m")
                nc.vector.reduce_max(m_sb, s_ps, axis=mybir.AxisListType.X)
                s2 = small.tile([QL, BPC, N], F32, tag="s2")
                nc.vector.tensor_sub(
                    s2,
                    s_ps,
                    m_sb[:]
                    .rearrange("p (b one) -> p b one", one=1)
                    .to_broadcast([QL, BPC, N]),
                )

                # top-16 per batch on s2 (same order as softmax weights)
                for b in range(BPC):
                    v16 = small.tile([QL, TOPK], F32, tag=f"v16_{b}")
                    w2 = small.tile([QL, N], F32, tag=f"w2_{b}")
                    nc.vector.max(out=v16[:, 0:8], in_=s2[:, b, :])
                    nc.vector.max_index(
                        i16_all[:, b, 0:8], v16[:, 0:8], s2[:, b, :]
                    )
                    nc.vector.match_replace(
                        out=w2,
                        in_to_replace=v16[:, 0:8],
                        in_values=s2[:, b, :],
                        imm_value=NEG,
                    )
                    nc.vector.max(out=v16[:, 8:16], in_=w2)
                    nc.vector.max_index(i16_all[:, b, 8:16], v16[:, 8:16], w2)
                nc.sync.dma_start(
                    out=idx_d.ap().rearrange("(b i) k -> i b k", b=BPC),
                    in_=i16_all,
                )

                # softmax tail: exp, per-batch sum, normalize
                w_sb = small.tile([QL, BPC, N], F32, tag="w")
                nc.scalar.activation(
                    w_sb,
                    s2,
                    mybir.ActivationFunctionType.Exp,
                    scale=float(SCALE),
                )
                sum_sb = small.tile([QL, BPC], F32, tag="sum")
                nc.vector.reduce_sum(sum_sb, w_sb, axis=mybir.AxisListType.X)
                rec = small.tile([QL, BPC], F32, tag="rec")
                nc.vector.reciprocal(rec, sum_sb)
                wn_all = small.tile([QL, BPC, N], F32, tag="wn")
                nc.vector.tensor_mul(
                    wn_all,
                    w_sb,
                    rec[:]
                    .rearrange("p (b one) -> p b one", one=1)
                    .to_broadcast([QL, BPC, N]),
                )
                nc.sync.dma_start(
                    out=attn_d.ap().rearrange("(b i) n -> i b n", b=BPC),
                    in_=wn_all,
                )
    nc.compile()
    return nc


def _get_nc(which, niter=1):
    key = (which, niter)
    if key not in _CACHE:
        nc = bacc.Bacc(
            trn_type="TRN2",
            target_bir_lowering=False,
            debug=False,
            num_devices=NCORES,
        )
        _CACHE[key] = (_emit_k1 if which == "k1" else _emit_k2)(nc, niter)
    return _CACHE[key]


def _split_bf16(a):
    """fp32 -> (hi, lo) bf16 pair with hi + lo ~= a (|err| <~ 2^-16 |a|)."""
    hi = a.astype(ml_dtypes.bfloat16)
    lo = (a - hi.astype(np.float32)).astype(ml_dtypes.bfloat16)
    return hi, lo


def _k1_in_maps(query, W_summary):
    # X[r, lh]: r = b*QL + i (b-major), lh = l*H + h
    x = np.ascontiguousarray(
        np.asarray(query, dtype=np.float32).transpose(2, 0, 1, 3)
    ).reshape(ROWS, LH)
    xt = np.ascontiguousarray(x.T)                     # [LH, 64]
    wt = np.ascontiguousarray(np.asarray(W_summary, dtype=np.float32).T)  # [LH, DK]
    xt_hl = _split_bf16(xt)
    wt_hl = _split_bf16(wt)
    in_maps = []
    for c in range(NCORES):
        m = {}
        for h in range(2):
            # xt packed partition-outer: [128, KT, ROWS]
            m[f"xt{h}"] = np.ascontiguousarray(
                xt_hl[h][c * KC : (c + 1) * KC]
                .reshape(2, KT // 2, 128, ROWS)
                .transpose(0, 2, 1, 3)
            )
            # wt packed partition-outer per WPAIR group
            m[f"wt{h}"] = np.ascontiguousarray(
                wt_hl[h][c * KC : (c + 1) * KC]
                .reshape(KT // WPAIR, WPAIR, 128, DK)
                .transpose(0, 2, 1, 3)
            )
        in_maps.append(m)
    return in_maps


def _k2_in_maps(qps, keys, b_summary):
    """qps: list of 8 per-core partial projections [64, 512] from launch 1.
    Pure gather/transpose glue - no arithmetic on values."""
    kt = np.ascontiguousarray(
        np.asarray(keys, dtype=np.float32).transpose(1, 2, 0)
    )  # [B, DK, N]
    bias = np.asarray(b_summary, dtype=np.float32)
    # bias block [p, t, r] = bias[t*128 + p], replicated over r
    biast = np.broadcast_to(
        bias.reshape(DKT, 128).T.reshape(128, 1, DKT, 1), (128, 1, DKT, RPC)
    )
    in_maps = []
    for c in range(NCORES):
        # parts[p, s, t, r]; s=0 is the bias block, s=1.. the partials
        srcs = np.stack(
            [
                np.asarray(qps[s])[c * RPC : (c + 1) * RPC].T.reshape(
                    DKT, 128, RPC
                )
                for s in range(NCORES)
            ]
        ).transpose(2, 0, 1, 3)
        parts = np.ascontiguousarray(
            np.concatenate([biast, srcs], axis=1)
        )
        # kt packed partition-outer: [128, BPC, DKT, N]
        ktc = np.ascontiguousarray(
            kt[c * BPC : (c + 1) * BPC]
            .reshape(BPC, DKT, 128, N)
            .transpose(2, 0, 1, 3)
        )
        in_maps.append({"parts": parts, "kt": ktc})
    return in_maps


def _unshard(results):
    """Per-core [RPC,N]/[RPC,TOPK] (row r'=b_local*QL+i) -> full outputs."""
    attn = np.empty((QL, B, N), dtype=np.float32)
    idx = np.empty((QL, B, TOPK), dtype=np.int32)
    for c in range(NCORES):
        a = np.asarray(results[c]["attn"]).reshape(BPC, QL, N)
        t = np.asarray(results[c]["idx"]).view(np.int32).reshape(BPC, QL, TOPK)
        attn[:, c * BPC : (c + 1) * BPC, :] = a.transpose(1, 0, 2)
        idx[:, c * BPC : (c + 1) * BPC, :] = t.transpose(1, 0, 2)
    attention = attn.reshape(ROWS, 1, N)
    topk_indices = np.ascontiguousarray(idx.reshape(ROWS, TOPK).T)
    return attention, topk_indices


def _run_spmd(nc, in_maps):
    return run_bass_kernel_spmd(nc, in_maps, core_ids=list(range(NCORES)))


def _kernel_impl(query, keys, W_summary, b_summary):
    res1 = _run_spmd(_get_nc("k1"), _k1_in_maps(query, W_summary))
    qps = [res1.results[c]["qp"] for c in range(NCORES)]
    res2 = _run_spmd(_get_nc("k2"), _k2_in_maps(qps, keys, b_summary))
    return _unshard(res2.results)


def _subproc_main(in_path, out_path):
    """Entry point for the fresh-process fallback (see kernel())."""
    import jax

    # touch each device with a plain op first - observed to help clear a
    # wedged exec unit left by a crashed predecessor session
    for dev in jax.devices()[:NCORES]:
        jax.device_put(np.zeros((1,), np.float32), dev).block_until_ready()
    d = np.load(in_path)
    attention, topk_indices = _kernel_impl(
        d["query"], d["keys"], d["W_summary"], d["b_summary"]
    )
    np.savez(out_path, attention=attention, topk_indices=topk_indices)


def _kernel_subprocess(query, keys, W_summary, b_summary):
    import subprocess
    import tempfile

    kdir = os.path.dirname(os.path.abspath(__file__))
    with tempfile.TemporaryDirectory() as td:
        in_path = os.path.join(td, "in.npz")
        out_path = os.path.join(td, "out.npz")
        np.savez(
            in_path,
            query=np.asarray(query, np.float32),
            keys=np.asarray(keys, np.float32),
            W_summary=np.asarray(W_summary, np.float32),
            b_summary=np.asarray(b_summary, np.float32),
        )
        env = dict(os.environ)
        env["PYTHONPATH"] = kdir + os.pathsep + env.get("PYTHONPATH", "")
        subprocess.run(
            [
                sys.executable,
                "-c",
                "import kernel; kernel._subproc_main(%r, %r)"
                % (in_path, out_path),
            ],
            env=env,
            check=True,
            timeout=900,
        )
        d = np.load(out_path)
        return d["attention"], d["topk_indices"]


def kernel(query, keys, values, W_summary, b_summary):
    del values  # dead in the reference: its einsum result is discarded
    # The axon/PJRT path occasionally throws a transient
    # NRT_EXEC_UNIT_UNRECOVERABLE on a session's first NEFF dispatch and the
    # failure poisons the whole jax session - only a fresh process recovers.
    # Fast path in-process; on failure fall back to fresh subprocesses.
    try:
        return _kernel_impl(query, keys, W_summary, b_summary)
    except Exception:
        last = None
        for _ in range(3):
            try:
                return _kernel_subprocess(query, keys, W_summary, b_summary)
            except Exception as e:  # noqa: PERF203
                last = e
                time.sleep(3.0)
        raise last


# revision 55
# speedup vs baseline: 1.0260x; 1.0076x over previous
"""Trainium2 Bass kernel for nn_Cache_65627100283720 (retrieval_knn).

Reference computation (jax):
    q = query.transpose(0,2,1,3).reshape(QL, B, L*H) @ W_summary.T + b_summary
    scores  = einsum('ibd,bnd->ibn', q, keys.transpose(1,0,2)) / sqrt(DK)
    weights = softmax(scores, -1)                      # -> attention [QL*B,1,N]
    topk_indices = top_k(weights, 16).T                # -> [16, QL*B]
(The big `values` einsum in the reference is dead code - its result is
discarded - so `values` never touches the device.)

Sharding (8 cores): the dominant cost is reading W_summary (32 MiB) and query
(16 MiB). We shard the L*H=16384 contraction dim: core c owns L-steps
[4c,4c+4) i.e. a 2048-slice, reads only W^T[2048c:2048c+2048] (4 MiB) and its
query slice (2 MiB), and computes a partial q-projection [64,512] (launch 1).
The host then re-shards those partials by batch (row order is b-major, so a
row-chunk == a batch shard) - pure gather/layout glue, no arithmetic - and
launch 2 sums the 8 partials on-device (3 tree adds), adds the bias, and runs
scores, softmax and top-16 (Max8/MatchReplace/FindIndex8) per core for its 2
batches. (A single-launch ReduceScatter design is blocked: intra-chip
collectives crash NRT under the axon/PJRT execution path.)

Perf notes (vs the instruction cost model; modeled ~22us + ~13us):
- the projection runs as bf16 hi+lo 3-pass matmuls (full PE rate, ~2^-16
  error) instead of quarter-rate fp32; precision suffices to reproduce the
  reference's top-k ordering exactly;
- all host-side layouts are packed partition-outer so every DMA line is a
  >=1 KiB contiguous run (full HBM bandwidth), and launch 1 is DMA-bound at
  ~13us (4.5 MiB/core at ~352 GB/s);
- a few throwaway matmuls warm the PE clock (HAM ramp) during the initial
  DMA window so the real matmuls run at 2.4 GHz;
- top-16 runs on the max-shifted scores (same order as softmax weights -
  softmax is monotone) so the DVE top-k chain overlaps the ACT exp chain.
"""

import os
import sys
import time

if "/opt/trn_rl_repo" not in sys.path:
    sys.path.insert(0, "/opt/trn_rl_repo")

import ml_dtypes
import numpy as np

import concourse.bacc as bacc
import concourse.mybir as mybir
import concourse.tile as tile
from concourse.bass_utils import run_bass_kernel_spmd

NCORES = 8
QL, L, B, H = 4, 32, 16, 512
N, DK = 128, 512
LH = L * H                 # 16384
KC = LH // NCORES          # 2048 contraction elems per core
KT = KC // 128             # 16 k-tiles per core
ROWS = QL * B              # 64 (row order: r = b*QL + i, b-major)
BPC = B // NCORES          # 2 batches per core
RPC = QL * BPC             # 8 rows per core
DKT = DK // 128            # 4 dk-tiles
TOPK = 16
SCALE = 1.0 / np.sqrt(np.float32(DK))
WPAIR = 2                  # k-tiles per W DMA transfer
NEG = -1.0e30              # below any score

F32 = mybir.dt.float32
BF16 = mybir.dt.bfloat16
U32 = mybir.dt.uint32

_CACHE = {}  # (which, niter) -> compiled Bacc program


def _body_iter(tc, niter):
    """niter >= 0: python-unrolled; niter < 0: tc.For_i hardware loop of
    -niter iterations (for loop-delta HW timing)."""
    if niter >= 0:
        yield from range(niter)
    else:
        with tc.For_i(0, -niter, 1):
            yield 0


def _warmup_pe(nc, pool, psum, n_mm=5, n_small=0):
    """Throwaway matmuls to ramp the PE clock while input DMAs run."""
    junk = pool.tile([128, N], F32, tag="warm_junk")
    nc.vector.memset(junk, 0.0)
    wps = psum.tile([128, N], F32, tag="warm_ps")
    for _ in range(n_mm):
        nc.tensor.matmul(wps, lhsT=junk[:], rhs=junk[:], start=True, stop=True)
    for _ in range(n_small):
        nc.tensor.matmul(
            wps[:, 0:64], lhsT=junk[:], rhs=junk[:, 0:64], start=True, stop=True
        )


def _emit_k1(nc, niter=1):
    """Partial projection: qp[64,512] = X_c^T-tiles^T @ W_c^T-tiles.

    fp32 operands are split hi+lo into bf16 on the host; each k-tile does
    3 full-rate bf16 passes (hi*hi + hi*lo + lo*hi, fp32 PSUM accumulate)
    instead of one quarter-rate fp32 matmul. Dropped lo*lo term is
    ~2^-16 relative - far below the fp32 path's own rounding noise."""
    # xt[h][g, p, k, r] (k-half groups g), wt[h][j, p, i, d]; h = hi/lo
    XG = 2  # xt k-groups
    KG = KT // XG
    xt_d = [
        nc.dram_tensor(f"xt{h}", [XG, 128, KG, ROWS], BF16, kind="ExternalInput")
        for h in range(2)
    ]
    wt_d = [
        nc.dram_tensor(
            f"wt{h}", [KT // WPAIR, 128, WPAIR, DK], BF16, kind="ExternalInput"
        )
        for h in range(2)
    ]
    qp_d = nc.dram_tensor("qp", [ROWS, DK], F32, kind="ExternalOutput")

    with tile.TileContext(nc) as tc:
        with (
            tc.tile_pool(name="xpool", bufs=4) as xpool,
            tc.tile_pool(name="wpool", bufs=16) as wpool,
            tc.tile_pool(name="opool", bufs=2) as opool,
            tc.tile_pool(name="psum", bufs=2, space="PSUM") as psum,
        ):
            _warmup_pe(nc, opool, psum, n_mm=5)
            for _ in _body_iter(tc, niter):
                xt_sb = {}

                def load_xt(g, h):
                    x_h = xpool.tile([128, KG, ROWS], BF16, tag=f"xt{h}")
                    nc.sync.dma_start(out=x_h, in_=xt_d[h][g])
                    xt_sb[h, g] = x_h

                load_xt(0, 0)
                # two dk-half accumulators: half A's copy/out overlaps the
                # final matmuls and copy of half B
                HDK = DK // 2
                qp_ps0 = psum.tile([ROWS, HDK], F32, tag="qp0")
                qp_ps1 = psum.tile([ROWS, HDK], F32, tag="qp1")
                qp_ps = [qp_ps0, qp_ps1]
                nmm_h = [0, 0]
                for j in range(KT // WPAIR):
                    wt_j = []
                    for h in range(2):
                        w_h = wpool.tile([128, WPAIR, DK], BF16, tag=f"wt{h}")
                        nc.sync.dma_start(out=w_h, in_=wt_d[h][j])
                        wt_j.append(w_h)
                        if j == 0 and h == 0:
                            # xt-lo lands after wt0-hi: the hi*hi pass can
                            # start as soon as xt-hi + wt0-hi are in
                            load_xt(0, 1)
                    if j == 2:
                        # group-b xt arrives behind W pairs 0-2, ahead of its
                        # first use at k = KG (pair KG/WPAIR)
                        load_xt(1, 0)
                        load_xt(1, 1)
                    last_pair = j == KT // WPAIR - 1

                    def emit(k, hh):
                        nonlocal nmm
                        g, kg = divmod(k, KG)
                        # hi*hi first: it only needs the hi transfers
                        for xh, wh in ((0, 0), (0, 1), (1, 0)):
                            nc.tensor.matmul(
                                qp_ps[hh],
                                lhsT=xt_sb[xh, g][:, kg, :],
                                rhs=wt_j[wh][:, i_of[k], hh * HDK : (hh + 1) * HDK],
                                start=(nmm_h[hh] == 0),
                                stop=(nmm_h[hh] == 3 * KT - 1),
                            )
                            nmm_h[hh] += 1

                    i_of = {j * WPAIR + i: i for i in range(WPAIR)}
                    ks = [j * WPAIR + i for i in range(WPAIR)]
                    if last_pair:
                        # close half A first so its copy/out overlaps B's tail
                        for k in ks:
                            emit(k, 0)
                        for k in ks:
                            emit(k, 1)
                    else:
                        for k in ks:
                            emit(k, 0)
                            emit(k, 1)
                qp_sb = opool.tile([ROWS, DK], F32, tag="qp_sb")
                for hh in range(2):
                    nc.vector.tensor_copy(
                        qp_sb[:, hh * HDK : (hh + 1) * HDK], qp_ps[hh]
                    )
                    nc.sync.dma_start(
                        out=qp_d[:, hh * HDK : (hh + 1) * HDK],
                        in_=qp_sb[:, hh * HDK : (hh + 1) * HDK],
                    )
    nc.compile()
    return nc


def _emit_k2(nc, niter=1):
    """Sum 8 partials + bias (already transposed by host glue), scores,
    softmax, top-16 for this core's 2 batches."""
    NSRC = NCORES + 1  # block 0 = bias, blocks 1..8 = per-core partials
    TRW = DKT * RPC    # one source block: (t, r) columns
    # parts[p, s, t, r]: dk%128 p, source block s, dk-tile t, row r
    parts_d = nc.dram_tensor(
        "parts", [128, NSRC, DKT, RPC], F32, kind="ExternalInput"
    )
    kt_d = nc.dram_tensor("kt", [128, BPC, DKT, N], F32, kind="ExternalInput")
    attn_d = nc.dram_tensor("attn", [RPC, N], F32, kind="ExternalOutput")
    idx_d = nc.dram_tensor("idx", [RPC, TOPK], U32, kind="ExternalOutput")

    with tile.TileContext(nc) as tc:
        with (
            tc.tile_pool(name="kpool", bufs=1) as kpool,
            tc.tile_pool(name="small", bufs=2) as small,
            tc.tile_pool(name="psum", bufs=2, space="PSUM") as psum,
        ):
            _warmup_pe(nc, small, psum, n_mm=5)
            kt_sb = kpool.tile([128, BPC, DKT, N], F32, tag="kt")

            for it in _body_iter(tc, niter):
                # qt layout: [128 (dk%128), (t, r)] with col = t*RPC + r
                parts_sb = small.tile([128, NSRC * TRW], F32, tag="parts")
                nc.sync.dma_start(
                    out=parts_sb[:].rearrange(
                        "p (s t r) -> p s t r", s=NSRC, t=DKT
                    ),
                    in_=parts_d.ap(),
                )
                if it == 0:
                    for b in range(BPC):
                        nc.sync.dma_start(
                            out=kt_sb[:, b], in_=kt_d[:, b]
                        )

                # tree-sum source blocks 1..8, then fold in bias block 0
                def blk(i, j):
                    return parts_sb[:, i * TRW : j * TRW]

                nc.vector.tensor_add(blk(1, 5), blk(1, 5), blk(5, 9))
                nc.vector.tensor_add(blk(1, 3), blk(1, 3), blk(3, 5))
                nc.vector.tensor_add(blk(1, 2), blk(1, 2), blk(2, 3))
                qt_sb = small.tile([128, TRW], F32, tag="qt")
                nc.vector.tensor_add(qt_sb, blk(0, 1), blk(1, 2))

                # scores for both batches into one [4, 2*128] PSUM bank
                i16_all = small.tile([QL, BPC, TOPK], U32, tag="i16")
                s_ps = psum.tile([QL, BPC, N], F32, tag="s_ps")
                for b in range(BPC):
                    for t in range(DKT):
                        nc.tensor.matmul(
                            s_ps[:, b, :],
                            lhsT=qt_sb[
                                :, t * RPC + b * QL : t * RPC + (b + 1) * QL
                            ],
                            rhs=kt_sb[:, b, t, :],
                            start=(t == 0),
                            stop=(t == DKT - 1),
                        )

                # s2 = s - rowmax (per batch segment), staged into SBUF; this
                # both matches the reference's stabilized softmax and makes
                # the DVE top-k ops read SBUF (cheaper than PSUM reads).
                m_sb = small.tile([QL, BPC], F32, tag="m")
                nc.vector.reduce_max(m_sb, s_ps, axis=mybir.AxisListType.X)
                s2 = small.tile([QL, BPC, N], F32, tag="s2")
                nc.vector.tensor_sub(
                    s2,
                    s_ps,
                    m_sb[:]
                    .rearrange("p (b one) -> p b one", one=1)
                    .to_broadcast([QL, BPC, N]),
                )

                # top-16 per batch on s2 (same order as softmax weights)
                for b in range(BPC):
                    v16 = small.tile([QL, TOPK], F32, tag=f"v16_{b}")
                    w2 = small.tile([QL, N], F32, tag=f"w2_{b}")
                    nc.vector.max(out=v16[:, 0:8], in_=s2[:, b, :])
                    nc.vector.max_index(
                        i16_all[:, b, 0:8], v16[:, 0:8], s2[:, b, :]
                    )
                    nc.vector.match_replace(
                        out=w2,
                        in_to_replace=v16[:, 0:8],
                        in_values=s2[:, b, :],
                        imm_value=NEG,
                    )
                    nc.vector.max(out=v16[:, 8:16], in_=w2)
                    nc.vector.max_index(i16_all[:, b, 8:16], v16[:, 8:16], w2)
                nc.sync.dma_start(
                    out=idx_d.ap().rearrange("(b i) k -> i b k", b=BPC),
                    in_=i16_all,
                )

                # softmax tail: exp, per-batch sum, normalize
                w_sb = small.tile([QL, BPC, N], F32, tag="w")
                nc.scalar.activation(
                    w_sb,
                    s2,
                    mybir.ActivationFunctionType.Exp,
                    scale=float(SCALE),
                )
                sum_sb = small.tile([QL, BPC], F32, tag="sum")
                nc.vector.reduce_sum(sum_sb, w_sb, axis=mybir.AxisListType.X)
                rec = small.tile([QL, BPC], F32, tag="rec")
                nc.vector.reciprocal(rec, sum_sb)
                wn_all = small.tile([QL, BPC, N], F32, tag="wn")
                nc.vector.tensor_mul(
                    wn_all,
                    w_sb,
                    rec[:]
                    .rearrange("p (b one) -> p b one", one=1)
                    .to_broadcast([QL, BPC, N]),
                )
                nc.sync.dma_start(
                    out=attn_d.ap().rearrange("(b i) n -> i b n", b=BPC),
                    in_=wn_all,
                )
    nc.compile()
    return nc


def _get_nc(which, niter=1):
    key = (which, niter)
    if key not in _CACHE:
        nc = bacc.Bacc(
            trn_type="TRN2",
            target_bir_lowering=False,
            debug=False,
            num_devices=NCORES,
        )
        _CACHE[key] = (_emit_k1 if which == "k1" else _emit_k2)(nc, niter)
    return _CACHE[key]


def _split_bf16(a):
    """fp32 -> (hi, lo) bf16 pair with hi + lo ~= a (|err| <~ 2^-16 |a|)."""
    hi = a.astype(ml_dtypes.bfloat16)
    lo = (a - hi.astype(np.float32)).astype(ml_dtypes.bfloat16)
    return hi, lo


def _k1_in_maps(query, W_summary):
    # X[r, lh]: r = b*QL + i (b-major), lh = l*H + h
    x = np.ascontiguousarray(
        np.asarray(query, dtype=np.float32).transpose(2, 0, 1, 3)
    ).reshape(ROWS, LH)
    xt = np.ascontiguousarray(x.T)                     # [LH, 64]
    wt = np.ascontiguousarray(np.asarray(W_summary, dtype=np.float32).T)  # [LH, DK]
    xt_hl = _split_bf16(xt)
    wt_hl = _split_bf16(wt)
    in_maps = []
    for c in range(NCORES):
        m = {}
        for h in range(2):
            # xt packed partition-outer: [128, KT, ROWS]
            m[f"xt{h}"] = np.ascontiguousarray(
                xt_hl[h][c * KC : (c + 1) * KC]
                .reshape(2, KT // 2, 128, ROWS)
                .transpose(0, 2, 1, 3)
            )
            # wt packed partition-outer per WPAIR group
            m[f"wt{h}"] = np.ascontiguousarray(
                wt_hl[h][c * KC : (c + 1) * KC]
                .reshape(KT // WPAIR, WPAIR, 128, DK)
                .transpose(0, 2, 1, 3)
            )
        in_maps.append(m)
    return in_maps


def _k2_in_maps(qps, keys, b_summary):
    """qps: list of 8 per-core partial projections [64, 512] from launch 1.
    Pure gather/transpose glue - no arithmetic on values."""
    kt = np.ascontiguousarray(
        np.asarray(keys, dtype=np.float32).transpose(1, 2, 0)
    )  # [B, DK, N]
    bias = np.asarray(b_summary, dtype=np.float32)
    # bias block [p, t, r] = bias[t*128 + p], replicated over r
    biast = np.broadcast_to(
        bias.reshape(DKT, 128).T.reshape(128, 1, DKT, 1), (128, 1, DKT, RPC)
    )
    in_maps = []
    for c in range(NCORES):
        # parts[p, s, t, r]; s=0 is the bias block, s=1.. the partials
        srcs = np.stack(
            [
                np.asarray(qps[s])[c * RPC : (c + 1) * RPC].T.reshape(
                    DKT, 128, RPC
                )
                for s in range(NCORES)
            ]
        ).transpose(2, 0, 1, 3)
        parts = np.ascontiguousarray(
            np.concatenate([biast, srcs], axis=1)
        )
        # kt packed partition-outer: [128, BPC, DKT, N]
        ktc = np.ascontiguousarray(
            kt[c * BPC : (c + 1) * BPC]
            .reshape(BPC, DKT, 128, N)
            .transpose(2, 0, 1, 3)
        )
        in_maps.append({"parts": parts, "kt": ktc})
    return in_maps


def _unshard(results):
    """Per-core [RPC,N]/[RPC,TOPK] (row r'=b_local*QL+i) -> full outputs."""
    attn = np.empty((QL, B, N), dtype=np.float32)
    idx = np.empty((QL, B, TOPK), dtype=np.int32)
    for c in range(NCORES):
        a = np.asarray(results[c]["attn"]).reshape(BPC, QL, N)
        t = np.asarray(results[c]["idx"]).view(np.int32).reshape(BPC, QL, TOPK)
        attn[:, c * BPC : (c + 1) * BPC, :] = a.transpose(1, 0, 2)
        idx[:, c * BPC : (c + 1) * BPC, :] = t.transpose(1, 0, 2)
    attention = attn.reshape(ROWS, 1, N)
    topk_indices = np.ascontiguousarray(idx.reshape(ROWS, TOPK).T)
    return attention, topk_indices


def _run_spmd(nc, in_maps):
    return run_bass_kernel_spmd(nc, in_maps, core_ids=list(range(NCORES)))


def _kernel_impl(query, keys, W_summary, b_summary):
    res1 = _run_spmd(_get_nc("k1"), _k1_in_maps(query, W_summary))
    qps = [res1.results[c]["qp"] for c in range(NCORES)]
    res2 = _run_spmd(_get_nc("k2"), _k2_in_maps(qps, keys, b_summary))
    return _unshard(res2.results)


def _subproc_main(in_path, out_path):
    """Entry point for the fresh-process fallback (see kernel())."""
    import jax

    # touch each device with a plain op first - observed to help clear a
    # wedged exec unit left by a crashed predecessor session
    for dev in jax.devices()[:NCORES]:
        jax.device_put(np.zeros((1,), np.float32), dev).block_until_ready()
    d = np.load(in_path)
    attention, topk_indices = _kernel_impl(
        d["query"], d["keys"], d["W_summary"], d["b_summary"]
    )
    np.savez(out_path, attention=attention, topk_indices=topk_indices)


def _kernel_subprocess(query, keys, W_summary, b_summary):
    import subprocess
    import tempfile

    kdir = os.path.dirname(os.path.abspath(__file__))
    with tempfile.TemporaryDirectory() as td:
        in_path = os.path.join(td, "in.npz")
        out_path = os.path.join(td, "out.npz")
        np.savez(
            in_path,
            query=np.asarray(query, np.float32),
            keys=np.asarray(keys, np.float32),
            W_summary=np.asarray(W_summary, np.float32),
            b_summary=np.asarray(b_summary, np.float32),
        )
        env = dict(os.environ)
        env["PYTHONPATH"] = kdir + os.pathsep + env.get("PYTHONPATH", "")
        subprocess.run(
            [
                sys.executable,
                "-c",
                "import kernel; kernel._subproc_main(%r, %r)"
                % (in_path, out_path),
            ],
            env=env,
            check=True,
            timeout=900,
        )
        d = np.load(out_path)
        return d["attention"], d["topk_indices"]


def kernel(query, keys, values, W_summary, b_summary):
    del values  # dead in the reference: its einsum result is discarded
    # The axon/PJRT path occasionally throws a transient
    # NRT_EXEC_UNIT_UNRECOVERABLE on a session's first NEFF dispatch and the
    # failure poisons the whole jax session - only a fresh process recovers.
    # Fast path in-process; on failure fall back to fresh subprocesses.
    try:
        return _kernel_impl(query, keys, W_summary, b_summary)
    except Exception:
        last = None
        for _ in range(3):
            try:
                return _kernel_subprocess(query, keys, W_summary, b_summary)
            except Exception as e:  # noqa: PERF203
                last = e
                time.sleep(3.0)
        raise last
